# revision 7
# baseline (speedup 1.0000x reference)
"""Multi-head GAT layer on 8 Trainium2 NeuronCores (Bass/Tile).

Strategy (dst-sharded, windowed-PSUM aggregation — no scatter):
- Each device owns a 6272-node slice of the 50000 destination nodes and
  receives exactly the edges pointing into its slice (host-bucketed).
- Phase Z (replicated): z = h @ [W | w_src | w_dst] -> z-table rows
  [z(256) | es | ed | pad] (stride 320 f32) in device HBM, plus a packed
  ed-pair table for 16-bit-indexable gathers by dst//2.
- Phase E: per 128-node window, gather z[src] rows (dma_gather, int16 idx,
  src-half split tables), build a 0/1 selection matrix (dst == node) on DVE,
  p = exp(leaky_relu(es_src + ed_dst)), msg = [p*z | p], and aggregate
  18 blocks x matmul(sel^T @ msg) into one PSUM tile. Normalizing by the
  aggregated p-sum reproduces softmax without any max-subtraction
  (logits are O(3), exp is safe).
- Output rows are written directly (distinct rows by construction); host
  concatenates the 8 slices.
"""

import numpy as np

N_NODES = 50000
N_EDGES = 800000
IN_DIM = 256
OUT_DIM = 64
H = 4
FEAT = H * OUT_DIM  # 256

ND = 8          # devices
OWN = 6272      # nodes owned per device (49 * 128)
NW = 49         # windows per device
BH = 9          # 128-slot blocks per (window, src-half)
WSLOT = 128 * BH  # 1152
NPAD = 50048    # padded node count (391 * 128)
NT = NPAD // 128  # 391 node tiles
ZCOLS = 320     # z-table row: z(256) es(4) ed(4) pad(56)
HALF = 25024    # src-half split point
PAIRS = NPAD // 2

_CACHED = {}


def _engine_of_pos(p):
    return 2 * ((p % 32) // 4) + (p // 64)


def _wrap_idx(a):
    """[..., n] int -> [..., 128, n/16] int16 (slot i at partition i%16, col i//16,
    replicated across the 8 16-partition groups)."""
    a = np.asarray(a)
    n = a.shape[-1]
    w = a.reshape(*a.shape[:-1], n // 16, 16)
    w = np.swapaxes(w, -1, -2).astype(np.int16)  # [..., 16, n/16]
    return np.tile(w, (1,) * (a.ndim - 1) + (8, 1))


def _build_program():
    if "nc" in _CACHED:
        return _CACHED["nc"]
    import concourse.bacc as bacc
    import concourse.mybir as mybir
    import concourse.tile as tile

    f32 = mybir.dt.float32
    i16 = mybir.dt.int16

    nc = bacc.Bacc("TRN2", num_swdge_queues=1)

    ht_d = nc.declare_dram_parameter("ht", [NT, 128, 2, 128], f32, isOutput=False)
    wf_d = nc.declare_dram_parameter("wf", [128, 2, 264], f32, isOutput=False)
    iota_d = nc.declare_dram_parameter("iota", [128, 128], f32, isOutput=False)
    zidx_d = nc.declare_dram_parameter("zidx", [NW, 2, 128, WSLOT // 16], i16, isOutput=False)
    edidx_d = nc.declare_dram_parameter("edidx", [NW, 2, 128, WSLOT // 16], i16, isOutput=False)
    # meta[w, half, p, 0, b] = ed parity of slot 128b+p; [.., 1, b] = rel dst (or -1 pad)
    meta_d = nc.declare_dram_parameter("meta", [NW, 2, 128, 2, BH], f32, isOutput=False)
    out_d = nc.declare_dram_parameter("out", [OWN, FEAT], f32, isOutput=True)

    zt_d = nc.dram_tensor("zt", [NPAD, ZCOLS], f32)
    edp_d = nc.dram_tensor("edp", [PAIRS, 128], f32)
    edp3 = edp_d.rearrange("n (a c) -> n a c", a=2)

    with tile.TileContext(nc) as tc:
        with (
            tc.tile_pool(name="const", bufs=1) as cpool,
            tc.tile_pool(name="zph", bufs=3) as zpool,
            tc.tile_pool(name="zpsum", bufs=2, space="PSUM") as zpsum,
            tc.tile_pool(name="edge", bufs=2) as epool,
            tc.tile_pool(name="small", bufs=3) as spool,
            tc.tile_pool(name="wpsum", bufs=2, space="PSUM") as wpsum,
        ):
            wft = cpool.tile([128, 2, 264], f32)
            nc.sync.dma_start(out=wft[:], in_=wf_d[:])
            iota = cpool.tile([128, 128], f32)
            nc.sync.dma_start(out=iota[:], in_=iota_d[:])

            # zero-init the ed-pair table (only cols 0:4 / 64:68 get written)
            zzt = cpool.tile([128, 128], f32)
            nc.vector.memset(zzt[:], 0.0)
            for r in range(0, PAIRS, 128):
                rr = min(128, PAIRS - r)
                nc.scalar.dma_start(out=edp_d[r:r + rr, :], in_=zzt[:rr, :])

            # ---------------- Phase Z ----------------
            for t in range(NT):
                kt = zpool.tile([128, 2, 128], f32, tag="kt")
                nc.sync.dma_start(out=kt[:], in_=ht_d[t])
                ps = zpsum.tile([128, 264], f32)
                nc.tensor.matmul(ps[:], lhsT=kt[:, 0, :], rhs=wft[:, 0, :],
                                 start=True, stop=False)
                nc.tensor.matmul(ps[:], lhsT=kt[:, 1, :], rhs=wft[:, 1, :],
                                 start=False, stop=True)
                zsb = zpool.tile([128, ZCOLS], f32, tag="zsb")
                nc.vector.tensor_copy(zsb[:, 0:264], ps[:])
                nc.vector.memset(zsb[:, 264:ZCOLS], 0.0)
                nc.sync.dma_start(out=zt_d[128 * t:128 * t + 128, :],
                                  in_=zsb[:])
                nc.scalar.dma_start(out=edp3[64 * t:64 * t + 64, :, 0:4],
                                    in_=zsb[:, 260:264])

            # ---------------- Phase E ----------------
            for w in range(NW):
                pw = wpsum.tile([128, 264], f32)
                for half in range(2):
                    zi = spool.tile([128, WSLOT // 16], i16, tag="zi")
                    nc.sync.dma_start(out=zi[:], in_=zidx_d[w, half])
                    ei = spool.tile([128, WSLOT // 16], i16, tag="ei")
                    nc.sync.dma_start(out=ei[:], in_=edidx_d[w, half])
                    mt = spool.tile([128, 2, BH], f32, tag="mt")
                    nc.sync.dma_start(out=mt[:], in_=meta_d[w, half])

                    zg = epool.tile([128, BH, ZCOLS], f32, tag="zg")
                    ztv = zt_d[0:HALF, :] if half == 0 else zt_d[HALF:NPAD, :]
                    nc.gpsimd.dma_gather(zg[:], ztv, zi[:], WSLOT, WSLOT, ZCOLS,
                                         single_packet=False, queue_num=0)
                    eg = epool.tile([128, BH, 128], f32, tag="eg")
                    nc.gpsimd.dma_gather(eg[:], edp_d[:, :], ei[:], WSLOT, WSLOT, 128,
                                         single_packet=False, queue_num=0)

                    # ed = ed_even + (ed_odd - ed_even) * parity
                    ed4 = spool.tile([128, BH, 4], f32, tag="ed4")
                    nc.vector.tensor_sub(ed4[:], eg[:, :, 64:68], eg[:, :, 0:4])
                    par = mt[:, 0, :, None].broadcast_to([128, BH, 4])
                    nc.vector.tensor_mul(ed4[:], ed4[:], par)
                    nc.vector.tensor_add(ed4[:], ed4[:], eg[:, :, 0:4])
                    # p = exp(leaky_relu(es + ed))
                    lg = spool.tile([128, BH, 4], f32, tag="lg")
                    nc.vector.tensor_add(lg[:], zg[:, :, 256:260], ed4[:])
                    p = spool.tile([128, BH, 4], f32, tag="p")
                    nc.vector.tensor_scalar_mul(p[:], lg[:], 0.01)
                    nc.vector.tensor_tensor(out=p[:], in0=lg[:], in1=p[:],
                                            op=mybir.AluOpType.max)
                    nc.scalar.activation(p[:], p[:],
                                         mybir.ActivationFunctionType.Exp)
                    # selection matrix: sel[s, b, n] = (reldst[s,b] == n)
                    sel = epool.tile([128, BH, 128], f32, tag="sel")
                    nc.vector.tensor_tensor(
                        out=sel[:],
                        in0=mt[:, 1, :, None].broadcast_to([128, BH, 128]),
                        in1=iota[:, None, :].broadcast_to([128, BH, 128]),
                        op=mybir.AluOpType.is_equal)
                    # msg = [p * z | p]
                    msg = epool.tile([128, BH, 260], f32, tag="msg")
                    nc.vector.tensor_mul(
                        msg[:, :, 0:256].rearrange("p b (h f) -> p b h f", h=H),
                        zg[:, :, 0:256].rearrange("p b (h f) -> p b h f", h=H),
                        p[:, :, :, None].broadcast_to([128, BH, H, OUT_DIM]))
                    nc.vector.tensor_copy(msg[:, :, 256:260], p[:])

                    for b in range(BH):
                        nc.tensor.matmul(pw[:, 0:260], lhsT=sel[:, b, :],
                                         rhs=msg[:, b, :],
                                         start=(half == 0 and b == 0),
                                         stop=(half == 1 and b == BH - 1))

                den = spool.tile([128, 4], f32, tag="den")
                nc.vector.tensor_scalar_add(den[:], pw[:, 256:260], 1e-30)
                rec = spool.tile([128, 4], f32, tag="rec")
                nc.vector.reciprocal(rec[:], den[:])
                osb = spool.tile([128, FEAT], f32, tag="osb")
                nc.vector.tensor_mul(
                    osb[:].rearrange("p (h f) -> p h f", h=H),
                    pw[:, 0:256].rearrange("p (h f) -> p h f", h=H),
                    rec[:, :, None].broadcast_to([128, H, OUT_DIM]))
                nc.sync.dma_start(out=out_d[128 * w:128 * w + 128, :], in_=osb[:])

    nc.compile()
    _CACHED["nc"] = nc
    return nc


def _prep_inputs(h, src, dst, W, a_src, a_dst):
    h = np.asarray(h, np.float32)
    src = np.asarray(src, np.int64)
    dst = np.asarray(dst, np.int64)
    W = np.asarray(W, np.float32)
    a_src = np.asarray(a_src, np.float32)
    a_dst = np.asarray(a_dst, np.float32)

    # Wfull [256, 264] = [W_cat | w_src | w_dst]
    wcat = W.transpose(1, 0, 2).reshape(IN_DIM, FEAT)
    w_src = np.einsum("hio,ho->ih", W, a_src)
    w_dst = np.einsum("hio,ho->ih", W, a_dst)
    wfull = np.concatenate([wcat, w_src, w_dst], axis=1)  # [256, 264]
    # wf[p, k, c] = wfull[128k + p, c]
    wf = np.ascontiguousarray(wfull.reshape(2, 128, 264).transpose(1, 0, 2))

    h_pad = np.zeros((NPAD, IN_DIM), np.float32)
    h_pad[:N_NODES] = h
    # ht[t, kp, k, m] = h_pad[128t + m, 128k + kp]
    ht = np.ascontiguousarray(
        h_pad.reshape(NT, 128, 2, 128).transpose(0, 3, 2, 1))

    iota = np.broadcast_to(np.arange(128, dtype=np.float32), (128, 128)).copy()

    # ---- bucket edges ----
    dev = dst // OWN
    half = (src >= HALF).astype(np.int64)
    win = (dst - dev * OWN) // 128
    rel = (dst - dev * OWN) % 128
    key = ((dev * NW + win) * 2 + half)
    order = np.argsort(key, kind="stable")
    ks = key[order]
    counts = np.bincount(ks, minlength=ND * NW * 2)
    assert counts.max() <= WSLOT, f"window overflow: {counts.max()} > {WSLOT}"
    starts = np.zeros(ND * NW * 2, np.int64)
    starts[1:] = np.cumsum(counts)[:-1]
    # position of each edge inside its (dev, win, half) bucket
    pos_in_bucket = np.arange(N_EDGES) - starts[ks]

    slot_base = ks * WSLOT + pos_in_bucket  # global slot id in [0, ND*NW*2*WSLOT)
    tot = ND * NW * 2 * WSLOT
    zidx = np.zeros(tot, np.int64)
    edidx = np.zeros(tot, np.int64)
    edpar = np.zeros(tot, np.float32)
    reldst = np.full(tot, -1.0, np.float32)

    so = order
    zidx[slot_base] = src[so] - half[so] * HALF
    edidx[slot_base] = dst[so] // 2
    edpar[slot_base] = (dst[so] % 2).astype(np.float32)
    reldst[slot_base] = rel[so].astype(np.float32)

    zidx = zidx.reshape(ND, NW, 2, WSLOT)
    edidx = edidx.reshape(ND, NW, 2, WSLOT)
    edpar = edpar.reshape(ND, NW, 2, BH, 128)
    reldst = reldst.reshape(ND, NW, 2, BH, 128)

    zidx_w = _wrap_idx(zidx.reshape(-1, WSLOT)).reshape(ND, NW, 2, 128, WSLOT // 16)
    edidx_w = _wrap_idx(edidx.reshape(-1, WSLOT)).reshape(ND, NW, 2, 128, WSLOT // 16)
    # meta[w, half, p, 0, b] = edpar slot 128b+p ; [.., 1, b] = reldst
    meta = np.stack([edpar.transpose(0, 1, 2, 4, 3),
                     reldst.transpose(0, 1, 2, 4, 3)], axis=4)  # [ND,NW,2,128,2,BH]
    meta = np.ascontiguousarray(meta, np.float32)

    in_maps = []
    for d in range(ND):
        in_maps.append({
            "ht": ht, "wf": wf, "iota": iota,
            "zidx": np.ascontiguousarray(zidx_w[d]),
            "edidx": np.ascontiguousarray(edidx_w[d]),
            "meta": meta[d],
        })
    return in_maps


def kernel(**inputs):
    from concourse.bass_utils import run_bass_kernel_spmd

    nc = _build_program()
    in_maps = _prep_inputs(**inputs)
    res = run_bass_kernel_spmd(nc, in_maps, core_ids=list(range(ND)))
    outs = [res.results[d]["out"] for d in range(ND)]
    full = np.concatenate(outs, axis=0)[:N_NODES]
    return np.ascontiguousarray(full)


if __name__ == "__main__":
    import reference as R
    inputs = R.setup_inputs()
    inputs = {k: np.asarray(v) for k, v in inputs.items()}
    out = kernel(**inputs)
    exp = np.asarray(R.reference(**{k: v for k, v in inputs.items()}))
    num = np.linalg.norm(out - exp)
    den = np.linalg.norm(exp)
    print("Relative error:", num / den)


# revision 10
# speedup vs baseline: 33.0437x; 33.0437x over previous
"""Multi-head GAT layer on 8 Trainium2 NeuronCores (Bass/Tile).

Strategy (dst-sharded, windowed-PSUM aggregation — no scatter):
- Each device owns a 6272-node slice of the 50000 destination nodes and
  receives exactly the edges pointing into its slice (host-bucketed).
- Phase Z (replicated): z = h @ [W | w_src | w_dst] -> z-table rows
  [z(256) | es | ed | pad] (stride 320 f32) in device HBM, plus a packed
  ed-pair table for 16-bit-indexable gathers by dst//2.
- Phase E: per 128-node window, gather z[src] rows (dma_gather, int16 idx,
  src-half split tables), build a 0/1 selection matrix (dst == node) on DVE,
  p = exp(leaky_relu(es_src + ed_dst)), msg = [p*z | p], and aggregate
  18 blocks x matmul(sel^T @ msg) into one PSUM tile. Normalizing by the
  aggregated p-sum reproduces softmax without any max-subtraction
  (logits are O(3), exp is safe).
- Output rows are written directly (distinct rows by construction); host
  concatenates the 8 slices.
"""

import numpy as np

N_NODES = 50000
N_EDGES = 800000
IN_DIM = 256
OUT_DIM = 64
H = 4
FEAT = H * OUT_DIM  # 256

ND = 8          # devices
OWN = 6272      # nodes owned per device (49 * 128)
NW = 49         # windows per device
BH = 9          # 128-slot blocks per (window, src-half)
WSLOT = 128 * BH  # 1152
NPAD = 50048    # padded node count (391 * 128)
NT = NPAD // 128  # 391 node tiles
ZCOLS = 320     # z-table row: z(256) es(4) ed(4) pad(56)
HALF = 25024    # src-half split point
PAIRS = NPAD // 2

_CACHED = {}


def _engine_of_pos(p):
    return 2 * ((p % 32) // 4) + (p // 64)


def _wrap_idx(a):
    """[..., n] int -> [..., 128, n/16] int16 (slot i at partition i%16, col i//16,
    replicated across the 8 16-partition groups)."""
    a = np.asarray(a)
    n = a.shape[-1]
    w = a.reshape(*a.shape[:-1], n // 16, 16)
    w = np.swapaxes(w, -1, -2).astype(np.int16)  # [..., 16, n/16]
    return np.tile(w, (1,) * (a.ndim - 1) + (8, 1))


def _build_program(reps=1):
    if ("nc", reps) in _CACHED:
        return _CACHED[("nc", reps)]
    import concourse.bacc as bacc
    import concourse.mybir as mybir
    import concourse.tile as tile

    f32 = mybir.dt.float32
    i16 = mybir.dt.int16

    nc = bacc.Bacc("TRN2", num_swdge_queues=1)

    ht_d = nc.declare_dram_parameter("ht", [NT, 128, 2, 128], f32, isOutput=False)
    wf_d = nc.declare_dram_parameter("wf", [128, 2, 264], f32, isOutput=False)
    iota_d = nc.declare_dram_parameter("iota", [128, 128], f32, isOutput=False)
    zidx_d = nc.declare_dram_parameter("zidx", [NW, 2, 128, WSLOT // 16], i16, isOutput=False)
    edidx_d = nc.declare_dram_parameter("edidx", [NW, 2, 128, WSLOT // 16], i16, isOutput=False)
    # meta[w, half, p, 0, b] = ed parity of slot 128b+p; [.., 1, b] = rel dst (or -1 pad)
    meta_d = nc.declare_dram_parameter("meta", [NW, 2, 128, 2, BH], f32, isOutput=False)
    out_d = nc.declare_dram_parameter("out", [OWN, FEAT], f32, isOutput=True)

    zt_d = nc.dram_tensor("zt", [NPAD, ZCOLS], f32)
    edp_d = nc.dram_tensor("edp", [PAIRS, 128], f32)
    edp3 = edp_d.rearrange("n (a c) -> n a c", a=2)

    with tile.TileContext(nc) as tc:
        with (
            tc.tile_pool(name="const", bufs=1) as cpool,
            tc.tile_pool(name="zph", bufs=3) as zpool,
            tc.tile_pool(name="zpsum", bufs=2, space="PSUM") as zpsum,
            tc.tile_pool(name="edge", bufs=2) as epool,
            tc.tile_pool(name="small", bufs=3) as spool,
            tc.tile_pool(name="wpsum", bufs=2, space="PSUM") as wpsum,
        ):
            wft = cpool.tile([128, 2, 264], f32)
            nc.sync.dma_start(out=wft[:], in_=wf_d[:])
            iota = cpool.tile([128, 128], f32)
            nc.sync.dma_start(out=iota[:], in_=iota_d[:])

            # zero-init the ed-pair table (only cols 0:4 / 64:68 get written)
            zzt = cpool.tile([128, 128], f32)
            nc.vector.memset(zzt[:], 0.0)
            for r in range(0, PAIRS, 128):
                rr = min(128, PAIRS - r)
                nc.scalar.dma_start(out=edp_d[r:r + rr, :], in_=zzt[:rr, :])

            for _rep in range(reps):
                _kernel_body(nc, tc, mybir, cpool, zpool, zpsum, epool, spool,
                             wpsum, wft, iota,
                             ht_d, zidx_d, edidx_d, meta_d, out_d, zt_d, edp_d,
                             edp3)

    nc.compile()
    _CACHED[("nc", reps)] = nc
    return nc


def _kernel_body(nc, tc, mybir, cpool, zpool, zpsum, epool, spool, wpsum,
                 wft, iota, ht_d, zidx_d, edidx_d, meta_d, out_d, zt_d, edp_d,
                 edp3):
    f32 = mybir.dt.float32
    i16 = mybir.dt.int16
    if True:
        if True:
            # ---------------- Phase Z ----------------
            for t in range(NT):
                kt = zpool.tile([128, 2, 128], f32, tag="kt")
                nc.sync.dma_start(out=kt[:], in_=ht_d[t])
                ps = zpsum.tile([128, 264], f32)
                nc.tensor.matmul(ps[:], lhsT=kt[:, 0, :], rhs=wft[:, 0, :],
                                 start=True, stop=False)
                nc.tensor.matmul(ps[:], lhsT=kt[:, 1, :], rhs=wft[:, 1, :],
                                 start=False, stop=True)
                zsb = zpool.tile([128, ZCOLS], f32, tag="zsb")
                nc.vector.tensor_copy(zsb[:, 0:264], ps[:])
                nc.vector.memset(zsb[:, 264:ZCOLS], 0.0)
                nc.sync.dma_start(out=zt_d[128 * t:128 * t + 128, :],
                                  in_=zsb[:])
                nc.scalar.dma_start(out=edp3[64 * t:64 * t + 64, :, 0:4],
                                    in_=zsb[:, 260:264])

            # ---------------- Phase E ----------------
            for w in range(NW):
                pw = wpsum.tile([128, 264], f32)
                for half in range(2):
                    zi = spool.tile([128, WSLOT // 16], i16, tag="zi")
                    nc.sync.dma_start(out=zi[:], in_=zidx_d[w, half])
                    ei = spool.tile([128, WSLOT // 16], i16, tag="ei")
                    nc.sync.dma_start(out=ei[:], in_=edidx_d[w, half])
                    mt = spool.tile([128, 2, BH], f32, tag="mt")
                    nc.sync.dma_start(out=mt[:], in_=meta_d[w, half])

                    zg = epool.tile([128, BH, ZCOLS], f32, tag="zg")
                    ztv = zt_d[0:HALF, :] if half == 0 else zt_d[HALF:NPAD, :]
                    nc.gpsimd.dma_gather(zg[:], ztv, zi[:], WSLOT, WSLOT, ZCOLS,
                                         single_packet=False, queue_num=0)
                    eg = epool.tile([128, BH, 128], f32, tag="eg")
                    nc.gpsimd.dma_gather(eg[:], edp_d[:, :], ei[:], WSLOT, WSLOT, 128,
                                         single_packet=False, queue_num=0)

                    # ed = ed_even + (ed_odd - ed_even) * parity
                    ed4 = spool.tile([128, BH, 4], f32, tag="ed4")
                    nc.vector.tensor_sub(ed4[:], eg[:, :, 64:68], eg[:, :, 0:4])
                    par = mt[:, 0, :, None].broadcast_to([128, BH, 4])
                    nc.vector.tensor_mul(ed4[:], ed4[:], par)
                    nc.vector.tensor_add(ed4[:], ed4[:], eg[:, :, 0:4])
                    # p = exp(leaky_relu(es + ed))
                    lg = spool.tile([128, BH, 4], f32, tag="lg")
                    nc.vector.tensor_add(lg[:], zg[:, :, 256:260], ed4[:])
                    p = spool.tile([128, BH, 4], f32, tag="p")
                    nc.vector.tensor_scalar_mul(p[:], lg[:], 0.01)
                    nc.vector.tensor_tensor(out=p[:], in0=lg[:], in1=p[:],
                                            op=mybir.AluOpType.max)
                    nc.scalar.activation(p[:], p[:],
                                         mybir.ActivationFunctionType.Exp)
                    # selection matrix: sel[s, b, n] = (reldst[s,b] == n)
                    sel = epool.tile([128, BH, 128], f32, tag="sel")
                    nc.vector.tensor_tensor(
                        out=sel[:],
                        in0=mt[:, 1, :, None].broadcast_to([128, BH, 128]),
                        in1=iota[:, None, :].broadcast_to([128, BH, 128]),
                        op=mybir.AluOpType.is_equal)
                    # msg = [p * z | p]
                    msg = epool.tile([128, BH, 260], f32, tag="msg")
                    nc.vector.tensor_mul(
                        msg[:, :, 0:256].rearrange("p b (h f) -> p b h f", h=H),
                        zg[:, :, 0:256].rearrange("p b (h f) -> p b h f", h=H),
                        p[:, :, :, None].broadcast_to([128, BH, H, OUT_DIM]))
                    nc.vector.tensor_copy(msg[:, :, 256:260], p[:])

                    for b in range(BH):
                        nc.tensor.matmul(pw[:, 0:260], lhsT=sel[:, b, :],
                                         rhs=msg[:, b, :],
                                         start=(half == 0 and b == 0),
                                         stop=(half == 1 and b == BH - 1))

                den = spool.tile([128, 4], f32, tag="den")
                nc.vector.tensor_scalar_add(den[:], pw[:, 256:260], 1e-30)
                rec = spool.tile([128, 4], f32, tag="rec")
                nc.vector.reciprocal(rec[:], den[:])
                osb = spool.tile([128, FEAT], f32, tag="osb")
                nc.vector.tensor_mul(
                    osb[:].rearrange("p (h f) -> p h f", h=H),
                    pw[:, 0:256].rearrange("p (h f) -> p h f", h=H),
                    rec[:, :, None].broadcast_to([128, H, OUT_DIM]))
                nc.sync.dma_start(out=out_d[128 * w:128 * w + 128, :], in_=osb[:])


def _prep_inputs(h, src, dst, W, a_src, a_dst):
    h = np.asarray(h, np.float32)
    src = np.asarray(src, np.int64)
    dst = np.asarray(dst, np.int64)
    W = np.asarray(W, np.float32)
    a_src = np.asarray(a_src, np.float32)
    a_dst = np.asarray(a_dst, np.float32)

    # Wfull [256, 264] = [W_cat | w_src | w_dst]
    wcat = W.transpose(1, 0, 2).reshape(IN_DIM, FEAT)
    w_src = np.einsum("hio,ho->ih", W, a_src)
    w_dst = np.einsum("hio,ho->ih", W, a_dst)
    wfull = np.concatenate([wcat, w_src, w_dst], axis=1)  # [256, 264]
    # wf[p, k, c] = wfull[128k + p, c]
    wf = np.ascontiguousarray(wfull.reshape(2, 128, 264).transpose(1, 0, 2))

    h_pad = np.zeros((NPAD, IN_DIM), np.float32)
    h_pad[:N_NODES] = h
    # ht[t, kp, k, m] = h_pad[128t + m, 128k + kp]
    ht = np.ascontiguousarray(
        h_pad.reshape(NT, 128, 2, 128).transpose(0, 3, 2, 1))

    iota = np.broadcast_to(np.arange(128, dtype=np.float32), (128, 128)).copy()

    # ---- bucket edges ----
    dev = dst // OWN
    half = (src >= HALF).astype(np.int64)
    win = (dst - dev * OWN) // 128
    rel = (dst - dev * OWN) % 128
    key = ((dev * NW + win) * 2 + half)
    order = np.argsort(key, kind="stable")
    ks = key[order]
    counts = np.bincount(ks, minlength=ND * NW * 2)
    assert counts.max() <= WSLOT, f"window overflow: {counts.max()} > {WSLOT}"
    starts = np.zeros(ND * NW * 2, np.int64)
    starts[1:] = np.cumsum(counts)[:-1]
    # position of each edge inside its (dev, win, half) bucket
    pos_in_bucket = np.arange(N_EDGES) - starts[ks]

    slot_base = ks * WSLOT + pos_in_bucket  # global slot id in [0, ND*NW*2*WSLOT)
    tot = ND * NW * 2 * WSLOT
    zidx = np.zeros(tot, np.int64)
    edidx = np.zeros(tot, np.int64)
    edpar = np.zeros(tot, np.float32)
    reldst = np.full(tot, -1.0, np.float32)

    so = order
    zidx[slot_base] = src[so] - half[so] * HALF
    edidx[slot_base] = dst[so] // 2
    edpar[slot_base] = (dst[so] % 2).astype(np.float32)
    reldst[slot_base] = rel[so].astype(np.float32)

    zidx = zidx.reshape(ND, NW, 2, WSLOT)
    edidx = edidx.reshape(ND, NW, 2, WSLOT)
    edpar = edpar.reshape(ND, NW, 2, BH, 128)
    reldst = reldst.reshape(ND, NW, 2, BH, 128)

    zidx_w = _wrap_idx(zidx.reshape(-1, WSLOT)).reshape(ND, NW, 2, 128, WSLOT // 16)
    edidx_w = _wrap_idx(edidx.reshape(-1, WSLOT)).reshape(ND, NW, 2, 128, WSLOT // 16)
    # meta[w, half, p, 0, b] = edpar slot 128b+p ; [.., 1, b] = reldst
    meta = np.stack([edpar.transpose(0, 1, 2, 4, 3),
                     reldst.transpose(0, 1, 2, 4, 3)], axis=4)  # [ND,NW,2,128,2,BH]
    meta = np.ascontiguousarray(meta, np.float32)

    in_maps = []
    for d in range(ND):
        in_maps.append({
            "ht": ht, "wf": wf, "iota": iota,
            "zidx": np.ascontiguousarray(zidx_w[d]),
            "edidx": np.ascontiguousarray(edidx_w[d]),
            "meta": meta[d],
        })
    return in_maps


def kernel(**inputs):
    from concourse.bass_utils import run_bass_kernel_spmd

    nc = _build_program()
    in_maps = _prep_inputs(**inputs)
    res = run_bass_kernel_spmd(nc, in_maps, core_ids=list(range(ND)))
    outs = [res.results[d]["out"] for d in range(ND)]
    full = np.concatenate(outs, axis=0)[:N_NODES]
    return np.ascontiguousarray(full)


if __name__ == "__main__":
    import reference as R
    inputs = R.setup_inputs()
    inputs = {k: np.asarray(v) for k, v in inputs.items()}
    out = kernel(**inputs)
    exp = np.asarray(R.reference(**{k: v for k, v in inputs.items()}))
    num = np.linalg.norm(out - exp)
    den = np.linalg.norm(exp)
    print("Relative error:", num / den)
